# revision 3
# baseline (speedup 1.0000x reference)
"""DeepSeek-style MLHA (multi-head latent attention) Trainium2 kernel, v2.

Shapes: x [B=2, L=2048, D=2048], HEADS=16, HD=128, KV_RANK=256, ATTN=2048.
  q = x @ Wq;  latent = rms_norm(x @ Wkv_down) * kv_norm_w;  kv = latent @ Wkv_up
  k, v = split(kv);  out = softmax_causal(q k^T / sqrt(128)) v;  y = out @ Wo

Sharding: batch x head-group split across 8 cores: core c handles batch c//4
and heads [4*(c%4), 4*(c%4)+4).  Each core computes a full-D o_proj partial
for its batch; the host sums the 4 partials per batch.

Device strategy (cost model: matmul time = N_free * pe_cycle * cpr; fp32r and
bf16 are 1.0 cpr, fp8e4 DoubleRow is 0.5 cpr with 2 packed K-tiles):
- q/latent projections run as fp8e4 DoubleRow matmuls with host-prepared
  hi/lo error compensation (x = x_hi + x_lo, W*32 = W_hi + W_lo; the three
  products hi*hi + hi*lo + lo*hi restore ~fp16 accuracy at 0.75x the fp32r
  PE cost).  Cross terms pack (x_hi*W_lo, x_lo*W_hi) as the two K-tiles of
  one DoubleRow matmul.
- attention (scores, exp, AV, softmax denominator) runs in bf16 with 256-wide
  x blocks; the causal mask is added into the scores PSUM via a small
  identity x mask-constant matmul (no gpsimd affine_select on the critical
  path).  The exp folds the k-side rms-norm rstd via a per-partition scale.
- softmax denominator l is accumulated with a ones-vector matmul; 1/l is
  broadcast across partitions on the (otherwise idle) gpsimd engine and the
  attention output is normalized pre-o_proj so the o_proj accumulates all 4
  heads in a single PSUM group.
- o_proj runs at the end (PSUM is fully used by attention: st double-buffer
  8KB + ao 4KB + l 4KB = 16KB), bf16 x bf16, plain copies + DMA out.
"""

import numpy as np
import ml_dtypes

import concourse.bacc as bacc
import concourse.mybir as mybir
import concourse.tile as tile
from concourse.bass_utils import run_bass_kernel_spmd

F32 = mybir.dt.float32
BF16 = mybir.dt.bfloat16
F8 = mybir.dt.float8e4
AF = mybir.ActivationFunctionType
ALU = mybir.AluOpType
DR = mybir.MatmulPerfMode.DoubleRow

B, L, D = 2, 2048, 2048
HEADS, HD, R = 16, 128, 256
NC_CORES = 8
HPC = 4                  # heads per core
DQ = HPC * HD            # 512 local q/k/v dims
KC = 16                  # 128-row chunks of hidden dim
XC = 4                   # 512-token phase-1 chunks
YB = 16                  # 128-row y chunks
XB = 8                   # 256-wide attention x blocks
EPS = 1e-6
WS = 32.0                # weight pre-scale before fp8 split
ISQHD = float(1.0 / np.sqrt(HD))

F8NP = ml_dtypes.float8_e4m3
BFNP = ml_dtypes.bfloat16


def _emit(tc):
    nc = tc.nc
    xhl_d = nc.dram_tensor("xhl", [128, KC, 2, L], F8, kind="ExternalInput").ap()
    wq_d = nc.dram_tensor("wq", [128, 2, KC, DQ], F8, kind="ExternalInput").ap()
    wk_d = nc.dram_tensor("wk", [128, 2, KC, R], F8, kind="ExternalInput").ap()
    wuk_d = nc.dram_tensor("wuk", [128, 2, DQ], BF16, kind="ExternalInput").ap()
    wuv_d = nc.dram_tensor("wuv", [128, 2, DQ], BF16, kind="ExternalInput").ap()
    wo_d = nc.dram_tensor("wo", [128, HPC, D], BF16, kind="ExternalInput").ap()
    mska_d = nc.dram_tensor("mska", [128, HPC, 128], BF16, kind="ExternalInput").ap()
    idn_d = nc.dram_tensor("idn", [128, 128], BF16, kind="ExternalInput").ap()
    out_d = nc.dram_tensor("out", [L, D], F32, kind="ExternalOutput").ap()

    constp = tc.alloc_tile_pool(name="constp", bufs=1)
    zero_b = constp.tile([128, 1], F32, name="zero_b")
    nc.vector.memset(zero_b, 0.0)
    eps_b = constp.tile([128, 1], F32, name="eps_b")
    nc.vector.memset(eps_b, EPS)
    ones_bf = constp.tile([128, 1], BF16, name="ones_bf")
    nc.vector.memset(ones_bf, 1.0)
    oinv_bf = constp.tile([128, 1], BF16, name="oinv_bf")
    nc.vector.memset(oinv_bf, 1.0 / R)
    onef = constp.tile([1, 1], F32, name="onef")
    nc.vector.memset(onef, 1.0)

    wp = tc.alloc_tile_pool(name="wp", bufs=1)
    wq_sb = wp.tile([128, 2, KC, DQ], F8, name="wq_sb")
    wk_sb = wp.tile([128, 2, KC, R], F8, name="wk_sb")
    wuk_sb = wp.tile([128, 2, DQ], BF16, name="wuk_sb")
    wuv_sb = wp.tile([128, 2, DQ], BF16, name="wuv_sb")
    wo_sb = wp.tile([128, HPC, D], BF16, name="wo_sb")
    mska_sb = wp.tile([128, HPC, 128], BF16, name="mska_sb")
    idn_sb = wp.tile([128, 128], BF16, name="idn_sb")

    mp = tc.alloc_tile_pool(name="mp", bufs=1)
    qT = mp.tile([128, HPC, L], BF16, name="qT")
    kT = mp.tile([128, HPC, L], BF16, name="kT")
    v_sb = mp.tile([128, YB, DQ], BF16, name="v_sb")
    latT = mp.tile([128, 2, L], BF16, name="latT")
    ao_nrm = mp.tile([128, HPC, L], BF16, name="ao_nrm")
    ms_sb = mp.tile([1, L], F32, name="ms_sb")
    rstd_p = mp.tile([128, YB], F32, name="rstd_p")
    rstd_s = mp.tile([128, YB], F32, name="rstd_s")

    # ---- input DMAs, interleaved so PE can start early ----
    xq = tc.alloc_tile_pool(name="xq", bufs=1)
    xts = [None] * XC

    def x_tile(xc):
        xts[xc] = xq.tile([128, KC, 2, 512], F8, tag="x", bufs=2, name=f"x{xc}")
        return xts[xc]

    xsl = lambda xc: slice(xc * 512, (xc + 1) * 512)
    nc.sync.dma_start(out=wq_sb[:, :, 0:8, :], in_=wq_d[:, :, 0:8, :])
    xt0 = x_tile(0)
    nc.sync.dma_start(out=xt0[:, 0:4, :, :], in_=xhl_d[:, 0:4, :, xsl(0)])
    nc.sync.dma_start(out=xt0[:, 4:8, :, :], in_=xhl_d[:, 4:8, :, xsl(0)])
    nc.sync.dma_start(out=wq_sb[:, :, 8:16, :], in_=wq_d[:, :, 8:16, :])
    nc.sync.dma_start(out=xt0[:, 8:16, :, :], in_=xhl_d[:, 8:16, :, xsl(0)])
    nc.sync.dma_start(out=wk_sb, in_=wk_d)
    xt1 = x_tile(1)
    nc.sync.dma_start(out=xt1, in_=xhl_d[:, :, :, xsl(1)])
    nc.gpsimd.dma_start(out=wuk_sb, in_=wuk_d)
    nc.gpsimd.dma_start(out=wuv_sb, in_=wuv_d)
    nc.gpsimd.dma_start(out=wo_sb, in_=wo_d)
    nc.gpsimd.dma_start(out=mska_sb, in_=mska_d)
    nc.gpsimd.dma_start(out=idn_sb, in_=idn_d)

    # ---- phase 1: qT = (Wq.T x.T)/32, latT = (Wkd.T x.T)/32, ms ----
    psA = tc.alloc_tile_pool(name="psA", bufs=1, space="PSUM")
    smp = tc.alloc_tile_pool(name="smp", bufs=1)

    def dr_mains(ps, w_sb, osl, xt, j0, j1, first, last):
        for j in range(j0, j1):
            nc.tensor.matmul(ps, lhsT=w_sb[:, 1, 2 * j:2 * j + 2, osl],
                             rhs=xt[:, 2 * j:2 * j + 2, 0, :],
                             start=(first and j == j0), stop=False, perf_mode=DR)

    def dr_crosses(ps, w_sb, osl, xt, k0, k1, last):
        for k in range(k0, k1):
            nc.tensor.matmul(ps, lhsT=w_sb[:, 0:2, k, osl],
                             rhs=xt[:, k, 0:2, :],
                             start=False, stop=(last and k == k1 - 1), perf_mode=DR)

    for xc in range(XC):
        xt = xts[xc]
        if xc >= 1 and xc + 1 < XC:  # prefetch next chunk
            nxt = x_tile(xc + 1)
            nc.sync.dma_start(out=nxt, in_=xhl_d[:, :, :, xsl(xc + 1)])
        halves = [(0, 8), (8, 16)] if xc == 0 else [(0, 16)]
        qps = [psA.tile([128, 512], F32, tag=f"q{oc}", bufs=1, name=f"q{xc}_{oc}")
               for oc in range(HPC)]
        for (k0, k1) in halves:
            for oc in range(HPC):
                osl = slice(oc * 128, (oc + 1) * 128)
                dr_mains(qps[oc], wq_sb, osl, xt, k0 // 2, k1 // 2, k0 == 0, False)
                dr_crosses(qps[oc], wq_sb, osl, xt, k0, k1, k1 == 16)
        for oc in range(HPC):
            nc.scalar.activation(qT[:, oc, xsl(xc)], qps[oc], AF.Copy,
                                 bias=0.0, scale=1.0 / WS)
        ms_ps = psA.tile([1, 512], F32, tag="ms", bufs=1, name=f"ms{xc}")
        for rc in range(2):
            rsl = slice(rc * 128, (rc + 1) * 128)
            ps = psA.tile([128, 512], F32, tag=f"lat{rc}", bufs=1, name=f"lat{xc}_{rc}")
            dr_mains(ps, wk_sb, rsl, xt, 0, 8, True, False)
            dr_crosses(ps, wk_sb, rsl, xt, 0, 16, True)
            nc.scalar.activation(latT[:, rc, xsl(xc)], ps, AF.Copy,
                                 bias=0.0, scale=1.0 / WS)
            sq = smp.tile([128, 512], BF16, tag="sq", bufs=2, name=f"sq{xc}_{rc}")
            nc.scalar.activation(sq, ps, AF.Square, bias=zero_b, scale=1.0 / WS)
            nc.tensor.matmul(ms_ps, lhsT=oinv_bf, rhs=sq,
                             start=(rc == 0), stop=(rc == 1), skip_group_check=True)
        nc.scalar.copy(ms_sb[0:1, xsl(xc)], ms_ps)

    # ---- rstd: transpose ms -> [128, YB], 1/sqrt(ms+eps) ----
    msT_ps = psA.tile([128, YB], F32, tag="msT", bufs=1)
    for j in range(YB):
        nc.tensor.matmul(msT_ps[:, j:j + 1],
                         lhsT=ms_sb[0:1, j * 128:(j + 1) * 128],
                         rhs=onef, start=True, stop=True, skip_group_check=True)
    t_p = smp.tile([128, YB], F32, tag="tp", bufs=1)
    nc.scalar.activation(t_p, msT_ps, AF.Sqrt, bias=eps_b, scale=1.0)
    nc.vector.reciprocal(rstd_p, t_p)
    nc.vector.tensor_scalar_mul(rstd_s, rstd_p, ISQHD)

    # ---- phase 2: kT = Wuk.T latT; v = (lat @ Wuv) * rstd ----
    for yc in range(4):
        ysl = slice(yc * 512, (yc + 1) * 512)
        for ec in range(HPC):
            ps = psA.tile([128, 512], F32, tag=f"q{ec}", bufs=1, name=f"k{yc}_{ec}")
            for rc in range(2):
                nc.tensor.matmul(ps, lhsT=wuk_sb[:, rc, ec * 128:(ec + 1) * 128],
                                 rhs=latT[:, rc, ysl], start=(rc == 0), stop=(rc == 1))
            nc.scalar.copy(kT[:, ec, ysl], ps)
        for j in range(4):
            yg = yc * 4 + j
            ps = psA.tile([128, DQ], F32, tag=f"lat{j % 2}", bufs=1, name=f"v{yg}")
            for rc in range(2):
                nc.tensor.matmul(ps, lhsT=latT[:, rc, yg * 128:(yg + 1) * 128],
                                 rhs=wuv_sb[:, rc, :], start=(rc == 0), stop=(rc == 1))
            nc.vector.tensor_scalar_mul(v_sb[:, yg, :], ps, rstd_p[:, yg:yg + 1])
    psA.release()
    smp.release()
    xq.release()

    # ---- attention: 256-wide x blocks, causal ----
    psC = tc.alloc_tile_pool(name="psC", bufs=1, space="PSUM")
    atp = tc.alloc_tile_pool(name="atp", bufs=1)
    for xb in range(XB):
        x0 = xb * 256
        nyc = 2 * xb + 2
        ao_ps = psC.tile([128, HPC, 256], F32, tag="ao", bufs=1, name=f"ao{xb}")
        l_ps = psC.tile([1, HPC, 256], F32, tag="l", bufs=1, name=f"l{xb}")
        for iy in range(nyc):
            st = psC.tile([128, HPC, 256], F32, tag="st", bufs=2, name=f"st{xb}_{iy}")
            at = atp.tile([128, HPC, 256], BF16, tag="at", bufs=5, name=f"at{xb}_{iy}")
            d1 = iy == 2 * xb
            d2 = iy == 2 * xb + 1
            ks = slice(iy * 128, (iy + 1) * 128)
            if d2:
                # upper diag block: x in [128,256) only; tri mask there
                for h in range(HPC):
                    nc.tensor.matmul(st[:, h, 128:256], lhsT=kT[:, h, ks],
                                     rhs=qT[:, h, x0 + 128:x0 + 256],
                                     start=True, stop=False, skip_group_check=True)
                for hp in range(2):
                    nc.tensor.matmul(st[:, 2 * hp:2 * hp + 2, 128:256], lhsT=idn_sb,
                                     rhs=mska_sb[:, 2 * hp:2 * hp + 2, :],
                                     start=False, stop=True, skip_group_check=True)
                nc.scalar.activation(at[:, :, 128:256], st[:, :, 128:256], AF.Exp,
                                     bias=zero_b, scale=rstd_s[:, iy:iy + 1])
                for h in range(HPC):
                    nc.tensor.matmul(ao_ps[:, h, 128:256],
                                     lhsT=v_sb[:, iy, h * 128:(h + 1) * 128],
                                     rhs=at[:, h, 128:256],
                                     start=False, stop=(iy == nyc - 1),
                                     skip_group_check=True)
                for hp in range(2):
                    nc.tensor.matmul(l_ps[0:1, 2 * hp:2 * hp + 2, 128:256],
                                     lhsT=ones_bf,
                                     rhs=at[:, 2 * hp:2 * hp + 2, 128:256],
                                     start=False, stop=(iy == nyc - 1),
                                     skip_group_check=True)
            else:
                if d1:
                    # lower diag block: tri mask on x in [0,128)
                    for h in range(HPC):
                        nc.tensor.matmul(st[:, h, 0:128], lhsT=kT[:, h, ks],
                                         rhs=qT[:, h, x0:x0 + 128],
                                         start=True, stop=False, skip_group_check=True)
                        nc.tensor.matmul(st[:, h, 128:256], lhsT=kT[:, h, ks],
                                         rhs=qT[:, h, x0 + 128:x0 + 256],
                                         start=True, stop=True, skip_group_check=True)
                    for hp in range(2):
                        nc.tensor.matmul(st[:, 2 * hp:2 * hp + 2, 0:128], lhsT=idn_sb,
                                         rhs=mska_sb[:, 2 * hp:2 * hp + 2, :],
                                         start=False, stop=True, skip_group_check=True)
                else:
                    for h in range(HPC):
                        nc.tensor.matmul(st[:, h, :], lhsT=kT[:, h, ks],
                                         rhs=qT[:, h, x0:x0 + 256],
                                         start=True, stop=True, skip_group_check=True)
                nc.scalar.activation(at, st, AF.Exp,
                                     bias=zero_b, scale=rstd_s[:, iy:iy + 1])
                for h in range(HPC):
                    nc.tensor.matmul(ao_ps[:, h, :],
                                     lhsT=v_sb[:, iy, h * 128:(h + 1) * 128],
                                     rhs=at[:, h, :], start=(iy == 0),
                                     stop=(iy == nyc - 1), skip_group_check=True)
                for hp in range(2):
                    nc.tensor.matmul(l_ps[0:1, 2 * hp:2 * hp + 2, :], lhsT=ones_bf,
                                     rhs=at[:, 2 * hp:2 * hp + 2, :],
                                     start=(iy == 0), stop=(iy == nyc - 1),
                                     skip_group_check=True)
        rcp = atp.tile([1, HPC, 256], F32, tag="rcp", bufs=2, name=f"rcp{xb}")
        nc.vector.reciprocal(rcp, l_ps)
        rl = atp.tile([128, HPC, 256], F32, tag="rl", bufs=2, name=f"rl{xb}")
        nc.gpsimd.partition_broadcast(rl, rcp)
        for h in range(HPC):
            nc.vector.tensor_mul(ao_nrm[:, h, x0:x0 + 256], ao_ps[:, h, :], rl[:, h, :])
    psC.release()
    atp.release()

    # ---- o_proj: y[x, d] = sum_h ao_nrm[:, h, x].T @ wo[:, h, d] ----
    psD = tc.alloc_tile_pool(name="psD", bufs=1, space="PSUM")
    op = tc.alloc_tile_pool(name="op", bufs=1)
    for xs in range(16):
        for mc in range(4):
            msl = slice(mc * 512, (mc + 1) * 512)
            wps = psD.tile([128, 512], F32, tag="w", bufs=4, name=f"w{xs}_{mc}")
            for h in range(HPC):
                nc.tensor.matmul(wps, lhsT=ao_nrm[:, h, xs * 128:(xs + 1) * 128],
                                 rhs=wo_sb[:, h, msl],
                                 start=(h == 0), stop=(h == HPC - 1))
            o = op.tile([128, 512], F32, tag="o", bufs=6, name=f"o{xs}_{mc}")
            if (xs * 4 + mc) % 2 == 0:
                nc.scalar.copy(o, wps)
            else:
                nc.vector.tensor_copy(o, wps)
            nc.sync.dma_start(out=out_d[xs * 128:(xs + 1) * 128, msl], in_=o)
    psD.release()
    op.release()
    mp.release()
    wp.release()
    constp.release()


_NC_CACHE = None


def _build():
    global _NC_CACHE
    if _NC_CACHE is None:
        nc = bacc.Bacc()
        with tile.TileContext(nc) as tc:
            _emit(tc)
        nc.compile()
        _NC_CACHE = nc
    return _NC_CACHE


def _hl(a):
    """fp8 hi/lo error-compensation split of a float32 array."""
    hi = a.astype(F8NP)
    lo = (a - hi.astype(np.float32)).astype(F8NP)
    return hi, lo


def make_in_maps(inputs):
    x = np.asarray(inputs["x"], np.float32)
    Wq = np.asarray(inputs["Wq"], np.float32)
    Wkd = np.asarray(inputs["Wkv_down"], np.float32)
    Wup = np.asarray(inputs["Wkv_up"], np.float32) * np.asarray(
        inputs["kv_norm_w"], np.float32)[:, None]
    Wo = np.asarray(inputs["Wo"], np.float32)

    xhl_b = []
    for b in range(B):
        xT = np.ascontiguousarray(x[b].T).reshape(KC, 128, L).transpose(1, 0, 2)
        hi, lo = _hl(xT)
        xhl_b.append(np.ascontiguousarray(np.stack([hi, lo], axis=2)))

    def wpack(W, ncol):  # [D, ncol] f32 -> [128, 2(lo,hi), KC, ncol] fp8
        Wt = W.reshape(KC, 128, ncol).transpose(1, 0, 2)
        hi, lo = _hl(Wt)
        return np.ascontiguousarray(np.stack([lo, hi], axis=1))

    parts = np.arange(128)
    mask = np.where(np.arange(128)[None, None, :] < parts[:, None, None],
                    np.float32(-30000.0), np.float32(0.0))
    mask = np.broadcast_to(mask, (128, HPC, 128))

    wq_g = [wpack(Wq[:, g * DQ:(g + 1) * DQ] * WS, DQ) for g in range(HPC)]
    wk_p = wpack(Wkd * WS, R)
    in_maps = []
    for c in range(NC_CORES):
        b, g = divmod(c, HPC)
        wuk = Wup[:, g * DQ:(g + 1) * DQ]
        wuv = Wup[:, HEADS * HD + g * DQ:HEADS * HD + (g + 1) * DQ]
        in_maps.append({
            "xhl": xhl_b[b],
            "wq": wq_g[g],
            "wk": wk_p,
            "wuk": np.ascontiguousarray(
                wuk.reshape(2, 128, DQ).transpose(1, 0, 2)).astype(BFNP),
            "wuv": np.ascontiguousarray(
                wuv.reshape(2, 128, DQ).transpose(1, 0, 2)).astype(BFNP),
            "wo": np.ascontiguousarray(
                Wo[g * DQ:(g + 1) * DQ, :].reshape(HPC, 128, D)
                .transpose(1, 0, 2)).astype(BFNP),
            "mska": np.ascontiguousarray(mask).astype(BFNP),
            "idn": np.eye(128, dtype=np.float32).astype(BFNP),
        })
    return in_maps


def kernel(x, Wq, Wkv_down, kv_norm_w, Wkv_up, Wo):
    in_maps = make_in_maps(dict(x=x, Wq=Wq, Wkv_down=Wkv_down,
                                kv_norm_w=kv_norm_w, Wkv_up=Wkv_up, Wo=Wo))
    nc = _build()
    res = run_bass_kernel_spmd(nc, in_maps, core_ids=list(range(NC_CORES)))
    outs = [r["out"].astype(np.float32) for r in res.results]
    y0 = outs[0] + outs[1] + outs[2] + outs[3]
    y1 = outs[4] + outs[5] + outs[6] + outs[7]
    return np.stack([y0, y1]).reshape(B, L, D)


# revision 7
# speedup vs baseline: 1.0019x; 1.0019x over previous
"""DeepSeek-style MLHA (multi-head latent attention) Trainium2 kernel, v2.

Shapes: x [B=2, L=2048, D=2048], HEADS=16, HD=128, KV_RANK=256, ATTN=2048.
  q = x @ Wq;  latent = rms_norm(x @ Wkv_down) * kv_norm_w;  kv = latent @ Wkv_up
  k, v = split(kv);  out = softmax_causal(q k^T / sqrt(128)) v;  y = out @ Wo

Sharding: batch x head-group split across 8 cores: core c handles batch c//4
and heads [4*(c%4), 4*(c%4)+4).  Each core computes a full-D o_proj partial
for its batch; the host sums the 4 partials per batch.

Device strategy (cost model: matmul time = N_free * pe_cycle * cpr; fp32r and
bf16 are 1.0 cpr, fp8e4 DoubleRow is 0.5 cpr with 2 packed K-tiles):
- q/latent projections run as fp8e4 DoubleRow matmuls with host-prepared
  hi/lo error compensation (x = x_hi + x_lo, W*32 = W_hi + W_lo; the three
  products hi*hi + hi*lo + lo*hi restore ~fp16 accuracy at 0.75x the fp32r
  PE cost).  Cross terms pack (x_hi*W_lo, x_lo*W_hi) as the two K-tiles of
  one DoubleRow matmul.
- attention (scores, exp, AV, softmax denominator) runs in bf16 with 256-wide
  x blocks; the causal mask is added into the scores PSUM via a small
  identity x mask-constant matmul (no gpsimd affine_select on the critical
  path).  The exp folds the k-side rms-norm rstd via a per-partition scale.
- softmax denominator l is accumulated with a ones-vector matmul; 1/l is
  broadcast across partitions on the (otherwise idle) gpsimd engine and the
  attention output is normalized pre-o_proj so the o_proj accumulates all 4
  heads in a single PSUM group.
- o_proj runs at the end (PSUM is fully used by attention: st double-buffer
  8KB + ao 4KB + l 4KB = 16KB), bf16 x bf16, plain copies + DMA out.
"""

import numpy as np
import ml_dtypes

import concourse.bacc as bacc
import concourse.mybir as mybir
import concourse.tile as tile
from concourse.bass_utils import run_bass_kernel_spmd

F32 = mybir.dt.float32
BF16 = mybir.dt.bfloat16
F8 = mybir.dt.float8e4
AF = mybir.ActivationFunctionType
ALU = mybir.AluOpType
DR = mybir.MatmulPerfMode.DoubleRow

B, L, D = 2, 2048, 2048
HEADS, HD, R = 16, 128, 256
NC_CORES = 8
HPC = 4                  # heads per core
DQ = HPC * HD            # 512 local q/k/v dims
KC = 16                  # 128-row chunks of hidden dim
XC = 4                   # 512-token phase-1 chunks
YB = 16                  # 128-row y chunks
XB = 8                   # 256-wide attention x blocks
EPS = 1e-6
WS = 32.0                # weight pre-scale before fp8 split
ISQHD = float(1.0 / np.sqrt(HD))

F8NP = ml_dtypes.float8_e4m3
BFNP = ml_dtypes.bfloat16


def _emit(tc):
    nc = tc.nc
    xhl_d = nc.dram_tensor("xhl", [128, KC, 2, L], F8, kind="ExternalInput").ap()
    wq_d = nc.dram_tensor("wq", [128, 2, KC, DQ], F8, kind="ExternalInput").ap()
    wk_d = nc.dram_tensor("wk", [128, 2, KC, R], F8, kind="ExternalInput").ap()
    wuk_d = nc.dram_tensor("wuk", [128, 2, DQ], BF16, kind="ExternalInput").ap()
    wuv_d = nc.dram_tensor("wuv", [128, 2, DQ], BF16, kind="ExternalInput").ap()
    wo_d = nc.dram_tensor("wo", [128, HPC, D], BF16, kind="ExternalInput").ap()
    mska_d = nc.dram_tensor("mska", [128, HPC, 128], BF16, kind="ExternalInput").ap()
    idn_d = nc.dram_tensor("idn", [128, 128], BF16, kind="ExternalInput").ap()
    out_d = nc.dram_tensor("out", [L, D], F32, kind="ExternalOutput").ap()

    constp = tc.alloc_tile_pool(name="constp", bufs=1)
    zero_b = constp.tile([128, 1], F32, name="zero_b")
    nc.vector.memset(zero_b, 0.0)
    eps_b = constp.tile([128, 1], F32, name="eps_b")
    nc.vector.memset(eps_b, EPS)
    ones_bf = constp.tile([128, 1], BF16, name="ones_bf")
    nc.vector.memset(ones_bf, 1.0)
    oinv_bf = constp.tile([128, 1], BF16, name="oinv_bf")
    nc.vector.memset(oinv_bf, 1.0 / R)
    onef = constp.tile([1, 1], F32, name="onef")
    nc.vector.memset(onef, 1.0)

    wp = tc.alloc_tile_pool(name="wp", bufs=1)
    wq_sb = wp.tile([128, 2, KC, DQ], F8, name="wq_sb")
    wk_sb = wp.tile([128, 2, KC, R], F8, name="wk_sb")
    wuk_sb = wp.tile([128, 2, DQ], BF16, name="wuk_sb")
    wuv_sb = wp.tile([128, 2, DQ], BF16, name="wuv_sb")
    wo_sb = wp.tile([128, HPC, D], BF16, name="wo_sb")
    mska_sb = wp.tile([128, HPC, 128], BF16, name="mska_sb")
    idn_sb = wp.tile([128, 128], BF16, name="idn_sb")

    mp = tc.alloc_tile_pool(name="mp", bufs=1)
    qT = mp.tile([128, HPC, L], BF16, name="qT")
    kT = mp.tile([128, HPC, L], BF16, name="kT")
    v_sb = mp.tile([128, YB, DQ], BF16, name="v_sb")
    latT = mp.tile([128, 2, L], BF16, name="latT")
    ao_nrm = mp.tile([128, HPC, L], BF16, name="ao_nrm")
    ms_sb = mp.tile([1, L], F32, name="ms_sb")
    rstd_p = mp.tile([128, YB], F32, name="rstd_p")
    rstd_s = mp.tile([128, YB], F32, name="rstd_s")

    # ---- input DMAs, interleaved so PE can start early ----
    xq = tc.alloc_tile_pool(name="xq", bufs=1)
    xts = [None] * XC

    def x_tile(xc):
        xts[xc] = xq.tile([128, KC, 2, 512], F8, tag="x", bufs=2, name=f"x{xc}")
        return xts[xc]

    xsl = lambda xc: slice(xc * 512, (xc + 1) * 512)
    nc.sync.dma_start(out=wq_sb[:, :, 0:8, :], in_=wq_d[:, :, 0:8, :])
    xt0 = x_tile(0)
    nc.sync.dma_start(out=xt0[:, 0:4, :, :], in_=xhl_d[:, 0:4, :, xsl(0)])
    nc.sync.dma_start(out=xt0[:, 4:8, :, :], in_=xhl_d[:, 4:8, :, xsl(0)])
    nc.sync.dma_start(out=wq_sb[:, :, 8:16, :], in_=wq_d[:, :, 8:16, :])
    nc.sync.dma_start(out=xt0[:, 8:16, :, :], in_=xhl_d[:, 8:16, :, xsl(0)])
    nc.sync.dma_start(out=wk_sb, in_=wk_d)
    xt1 = x_tile(1)
    nc.sync.dma_start(out=xt1, in_=xhl_d[:, :, :, xsl(1)])
    nc.gpsimd.dma_start(out=wuk_sb, in_=wuk_d)
    nc.gpsimd.dma_start(out=wuv_sb, in_=wuv_d)
    nc.gpsimd.dma_start(out=wo_sb, in_=wo_d)
    nc.gpsimd.dma_start(out=mska_sb, in_=mska_d)
    nc.gpsimd.dma_start(out=idn_sb, in_=idn_d)

    # ---- phase 1: qT = (Wq.T x.T)/32, latT = (Wkd.T x.T)/32, ms ----
    psA = tc.alloc_tile_pool(name="psA", bufs=1, space="PSUM")
    smp = tc.alloc_tile_pool(name="smp", bufs=1)

    def dr_mains(ps, w_sb, osl, xt, j0, j1, first, last):
        for j in range(j0, j1):
            nc.tensor.matmul(ps, lhsT=w_sb[:, 1, 2 * j:2 * j + 2, osl],
                             rhs=xt[:, 2 * j:2 * j + 2, 0, :],
                             start=(first and j == j0), stop=False, perf_mode=DR)

    def dr_crosses(ps, w_sb, osl, xt, k0, k1, last):
        for k in range(k0, k1):
            nc.tensor.matmul(ps, lhsT=w_sb[:, 0:2, k, osl],
                             rhs=xt[:, k, 0:2, :],
                             start=False, stop=(last and k == k1 - 1), perf_mode=DR)

    for xc in range(XC):
        xt = xts[xc]
        if xc >= 1 and xc + 1 < XC:  # prefetch next chunk
            nxt = x_tile(xc + 1)
            nc.sync.dma_start(out=nxt, in_=xhl_d[:, :, :, xsl(xc + 1)])
        halves = [(0, 8), (8, 16)] if xc == 0 else [(0, 16)]
        qps = [psA.tile([128, 512], F32, tag=f"q{oc}", bufs=1, name=f"q{xc}_{oc}")
               for oc in range(HPC)]
        for (k0, k1) in halves:
            for oc in range(HPC):
                osl = slice(oc * 128, (oc + 1) * 128)
                dr_mains(qps[oc], wq_sb, osl, xt, k0 // 2, k1 // 2, k0 == 0, False)
                dr_crosses(qps[oc], wq_sb, osl, xt, k0, k1, k1 == 16)
        for oc in range(HPC):
            nc.scalar.activation(qT[:, oc, xsl(xc)], qps[oc], AF.Copy,
                                 bias=0.0, scale=1.0 / WS)
        ms_ps = psA.tile([1, 512], F32, tag="ms", bufs=1, name=f"ms{xc}")
        for rc in range(2):
            rsl = slice(rc * 128, (rc + 1) * 128)
            ps = psA.tile([128, 512], F32, tag=f"lat{rc}", bufs=1, name=f"lat{xc}_{rc}")
            dr_mains(ps, wk_sb, rsl, xt, 0, 8, True, False)
            dr_crosses(ps, wk_sb, rsl, xt, 0, 16, True)
            nc.scalar.activation(latT[:, rc, xsl(xc)], ps, AF.Copy,
                                 bias=0.0, scale=1.0 / WS)
            sq = smp.tile([128, 512], BF16, tag="sq", bufs=2, name=f"sq{xc}_{rc}")
            nc.scalar.activation(sq, ps, AF.Square, bias=zero_b, scale=1.0 / WS)
            nc.tensor.matmul(ms_ps, lhsT=oinv_bf, rhs=sq,
                             start=(rc == 0), stop=(rc == 1), skip_group_check=True)
        nc.scalar.copy(ms_sb[0:1, xsl(xc)], ms_ps)

    # ---- rstd: transpose ms -> [128, YB], 1/sqrt(ms+eps) ----
    msT_ps = psA.tile([128, YB], F32, tag="msT", bufs=1)
    for j in range(YB):
        nc.tensor.matmul(msT_ps[:, j:j + 1],
                         lhsT=ms_sb[0:1, j * 128:(j + 1) * 128],
                         rhs=onef, start=True, stop=True, skip_group_check=True)
    t_p = smp.tile([128, YB], F32, tag="tp", bufs=1)
    nc.scalar.activation(t_p, msT_ps, AF.Sqrt, bias=eps_b, scale=1.0)
    nc.vector.reciprocal(rstd_p, t_p)
    nc.vector.tensor_scalar_mul(rstd_s, rstd_p, ISQHD)

    # ---- phase 2: kT = Wuk.T latT; v = (lat @ Wuv) * rstd ----
    for yc in range(4):
        ysl = slice(yc * 512, (yc + 1) * 512)
        for ec in range(HPC):
            ps = psA.tile([128, 512], F32, tag=f"q{ec}", bufs=1, name=f"k{yc}_{ec}")
            for rc in range(2):
                nc.tensor.matmul(ps, lhsT=wuk_sb[:, rc, ec * 128:(ec + 1) * 128],
                                 rhs=latT[:, rc, ysl], start=(rc == 0), stop=(rc == 1))
            nc.scalar.copy(kT[:, ec, ysl], ps)
        for j in range(4):
            yg = yc * 4 + j
            ps = psA.tile([128, DQ], F32, tag=f"lat{j % 2}", bufs=1, name=f"v{yg}")
            for rc in range(2):
                nc.tensor.matmul(ps, lhsT=latT[:, rc, yg * 128:(yg + 1) * 128],
                                 rhs=wuv_sb[:, rc, :], start=(rc == 0), stop=(rc == 1))
            nc.vector.tensor_scalar_mul(v_sb[:, yg, :], ps, rstd_p[:, yg:yg + 1])
    psA.release()
    smp.release()
    xq.release()

    # ---- attention: 256-wide x blocks, causal ----
    psC = tc.alloc_tile_pool(name="psC", bufs=1, space="PSUM")
    atp = tc.alloc_tile_pool(name="atp", bufs=1)
    for xb in range(XB):
        x0 = xb * 256
        nyc = 2 * xb + 2
        ao_ps = psC.tile([128, HPC, 256], F32, tag="ao", bufs=1, name=f"ao{xb}")
        l_ps = psC.tile([1, HPC, 256], F32, tag="l", bufs=1, name=f"l{xb}")
        for iy in range(nyc):
            st = psC.tile([128, HPC, 256], F32, tag="st", bufs=2, name=f"st{xb}_{iy}")
            at = atp.tile([128, HPC, 256], BF16, tag="at", bufs=5, name=f"at{xb}_{iy}")
            d1 = iy == 2 * xb
            d2 = iy == 2 * xb + 1
            ks = slice(iy * 128, (iy + 1) * 128)
            if d2:
                # upper diag block: x in [128,256) only; tri mask there
                for h in range(HPC):
                    nc.tensor.matmul(st[:, h, 128:256], lhsT=kT[:, h, ks],
                                     rhs=qT[:, h, x0 + 128:x0 + 256],
                                     start=(h % 2 == 0), stop=False,
                                     skip_group_check=True)
                for h in range(HPC):
                    nc.tensor.matmul(st[:, h, 128:256], lhsT=idn_sb,
                                     rhs=mska_sb[:, h, :],
                                     start=False, stop=True, skip_group_check=True)
                nc.scalar.activation(at[:, :, 128:256], st[:, :, 128:256], AF.Exp,
                                     bias=zero_b, scale=rstd_s[:, iy:iy + 1])
                for h in range(HPC):
                    nc.tensor.matmul(ao_ps[:, h, 128:256],
                                     lhsT=v_sb[:, iy, h * 128:(h + 1) * 128],
                                     rhs=at[:, h, 128:256],
                                     start=False, stop=(iy == nyc - 1),
                                     skip_group_check=True)
                for h in range(HPC):
                    nc.tensor.matmul(l_ps[0:1, h, 128:256],
                                     lhsT=ones_bf,
                                     rhs=at[:, h, 128:256],
                                     start=False, stop=(iy == nyc - 1),
                                     skip_group_check=True)
            else:
                if d1:
                    # lower diag block: tri mask on x in [0,128)
                    for h in range(HPC):
                        nc.tensor.matmul(st[:, h, 0:128], lhsT=kT[:, h, ks],
                                         rhs=qT[:, h, x0:x0 + 128],
                                         start=(h % 2 == 0), stop=False,
                                         skip_group_check=True)
                        nc.tensor.matmul(st[:, h, 128:256], lhsT=kT[:, h, ks],
                                         rhs=qT[:, h, x0 + 128:x0 + 256],
                                         start=False, stop=True, skip_group_check=True)
                    for h in range(HPC):
                        nc.tensor.matmul(st[:, h, 0:128], lhsT=idn_sb,
                                         rhs=mska_sb[:, h, :],
                                         start=False, stop=True, skip_group_check=True)
                else:
                    for h in range(HPC):
                        nc.tensor.matmul(st[:, h, :], lhsT=kT[:, h, ks],
                                         rhs=qT[:, h, x0:x0 + 256],
                                         start=(h % 2 == 0), stop=True,
                                         skip_group_check=True)
                nc.scalar.activation(at, st, AF.Exp,
                                     bias=zero_b, scale=rstd_s[:, iy:iy + 1])
                for h in range(HPC):
                    nc.tensor.matmul(ao_ps[:, h, :],
                                     lhsT=v_sb[:, iy, h * 128:(h + 1) * 128],
                                     rhs=at[:, h, :],
                                     start=(iy == 0 and h % 2 == 0),
                                     stop=(iy == nyc - 1), skip_group_check=True)
                for hp in range(2):
                    nc.tensor.matmul(l_ps[0:1, 2 * hp:2 * hp + 2, :], lhsT=ones_bf,
                                     rhs=at[:, 2 * hp:2 * hp + 2, :],
                                     start=(iy == 0), stop=(iy == nyc - 1),
                                     skip_group_check=True)
        rcp = atp.tile([1, HPC, 256], F32, tag="rcp", bufs=2, name=f"rcp{xb}")
        nc.vector.reciprocal(rcp, l_ps)
        rl = atp.tile([128, HPC, 256], F32, tag="rl", bufs=2, name=f"rl{xb}")
        nc.gpsimd.partition_broadcast(rl, rcp)
        for h in range(HPC):
            nc.vector.tensor_mul(ao_nrm[:, h, x0:x0 + 256], ao_ps[:, h, :], rl[:, h, :])
    psC.release()
    atp.release()

    # ---- o_proj: y[x, d] = sum_h ao_nrm[:, h, x].T @ wo[:, h, d] ----
    psD = tc.alloc_tile_pool(name="psD", bufs=1, space="PSUM")
    op = tc.alloc_tile_pool(name="op", bufs=1)
    for xs in range(16):
        for mc in range(4):
            msl = slice(mc * 512, (mc + 1) * 512)
            wps = psD.tile([128, 512], F32, tag="w", bufs=4, name=f"w{xs}_{mc}")
            for h in range(HPC):
                nc.tensor.matmul(wps, lhsT=ao_nrm[:, h, xs * 128:(xs + 1) * 128],
                                 rhs=wo_sb[:, h, msl],
                                 start=(h == 0), stop=(h == HPC - 1))
            o = op.tile([128, 512], F32, tag="o", bufs=6, name=f"o{xs}_{mc}")
            if (xs * 4 + mc) % 2 == 0:
                nc.scalar.copy(o, wps)
            else:
                nc.vector.tensor_copy(o, wps)
            nc.sync.dma_start(out=out_d[xs * 128:(xs + 1) * 128, msl], in_=o)
    psD.release()
    op.release()
    mp.release()
    wp.release()
    constp.release()


_NC_CACHE = None


def _build():
    global _NC_CACHE
    if _NC_CACHE is None:
        nc = bacc.Bacc()
        with tile.TileContext(nc) as tc:
            _emit(tc)
        nc.compile()
        _NC_CACHE = nc
    return _NC_CACHE


def _hl(a):
    """fp8 hi/lo error-compensation split of a float32 array."""
    hi = a.astype(F8NP)
    lo = (a - hi.astype(np.float32)).astype(F8NP)
    return hi, lo


def make_in_maps(inputs):
    x = np.asarray(inputs["x"], np.float32)
    Wq = np.asarray(inputs["Wq"], np.float32)
    Wkd = np.asarray(inputs["Wkv_down"], np.float32)
    Wup = np.asarray(inputs["Wkv_up"], np.float32) * np.asarray(
        inputs["kv_norm_w"], np.float32)[:, None]
    Wo = np.asarray(inputs["Wo"], np.float32)

    xhl_b = []
    for b in range(B):
        xT = np.ascontiguousarray(x[b].T).reshape(KC, 128, L).transpose(1, 0, 2)
        hi, lo = _hl(xT)
        xhl_b.append(np.ascontiguousarray(np.stack([hi, lo], axis=2)))

    def wpack(W, ncol):  # [D, ncol] f32 -> [128, 2(lo,hi), KC, ncol] fp8
        Wt = W.reshape(KC, 128, ncol).transpose(1, 0, 2)
        hi, lo = _hl(Wt)
        return np.ascontiguousarray(np.stack([lo, hi], axis=1))

    parts = np.arange(128)
    mask = np.where(np.arange(128)[None, None, :] < parts[:, None, None],
                    np.float32(-30000.0), np.float32(0.0))
    mask = np.broadcast_to(mask, (128, HPC, 128))

    wq_g = [wpack(Wq[:, g * DQ:(g + 1) * DQ] * WS, DQ) for g in range(HPC)]
    wk_p = wpack(Wkd * WS, R)
    in_maps = []
    for c in range(NC_CORES):
        b, g = divmod(c, HPC)
        wuk = Wup[:, g * DQ:(g + 1) * DQ]
        wuv = Wup[:, HEADS * HD + g * DQ:HEADS * HD + (g + 1) * DQ]
        in_maps.append({
            "xhl": xhl_b[b],
            "wq": wq_g[g],
            "wk": wk_p,
            "wuk": np.ascontiguousarray(
                wuk.reshape(2, 128, DQ).transpose(1, 0, 2)).astype(BFNP),
            "wuv": np.ascontiguousarray(
                wuv.reshape(2, 128, DQ).transpose(1, 0, 2)).astype(BFNP),
            "wo": np.ascontiguousarray(
                Wo[g * DQ:(g + 1) * DQ, :].reshape(HPC, 128, D)
                .transpose(1, 0, 2)).astype(BFNP),
            "mska": np.ascontiguousarray(mask).astype(BFNP),
            "idn": np.eye(128, dtype=np.float32).astype(BFNP),
        })
    return in_maps


def kernel(x, Wq, Wkv_down, kv_norm_w, Wkv_up, Wo):
    in_maps = make_in_maps(dict(x=x, Wq=Wq, Wkv_down=Wkv_down,
                                kv_norm_w=kv_norm_w, Wkv_up=Wkv_up, Wo=Wo))
    nc = _build()
    res = run_bass_kernel_spmd(nc, in_maps, core_ids=list(range(NC_CORES)))
    outs = [r["out"].astype(np.float32) for r in res.results]
    y0 = outs[0] + outs[1] + outs[2] + outs[3]
    y1 = outs[4] + outs[5] + outs[6] + outs[7]
    return np.stack([y0, y1]).reshape(B, L, D)


# revision 8
# speedup vs baseline: 1.0713x; 1.0693x over previous
"""DeepSeek-style MLHA (multi-head latent attention) Trainium2 kernel, v3.

Shapes: x [B=2, L=2048, D=2048], HEADS=16, HD=128, KV_RANK=256, ATTN=2048.
  q = x @ Wq;  latent = rms_norm(x @ Wkv_down) * kv_norm_w;  kv = latent @ Wkv_up
  k, v = split(kv);  out = softmax_causal(q k^T / sqrt(128)) v;  y = out @ Wo

Sharding: batch x head-group split across 8 cores: core c handles batch c//4
and heads [4*(c%4), 4*(c%4)+4).  Each core computes a full-D o_proj partial
for its batch; the host sums the 4 partials per batch.

Device strategy (cost model: matmul time = N_free * pe_cycle * cpr; fp32r and
bf16 are 1.0 cpr, fp8e4 DoubleRow is 0.5 cpr with 2 packed K-tiles):
- q/latent/o projections run as fp8e4 DoubleRow matmuls with hi/lo error
  compensation (a = a_hi + a_lo, W*32 = W_hi + W_lo; hi*hi + hi*lo + lo*hi
  restores ~fp16 accuracy at 0.75x the fp32r PE cost).  Cross terms pack
  (a_hi*W_lo, a_lo*W_hi) as the two K-tiles of one DoubleRow matmul.
- attention (scores, exp, AV, softmax denominator) runs in bf16 with 256-wide
  x blocks; the causal mask is added into the scores PSUM via a small
  identity x mask-constant matmul.  The exp folds the k-side rms-norm rstd
  via a per-partition scale.  Scores for block iy+1 are emitted before the
  AV of block iy so the PE never waits on the exp.
- softmax denominator l: ones-vector matmul accumulation; at each x-block end
  the PSUM slots are freed fast (plain DVE copies of ao, gpsimd partition
  broadcast of l) and the normalize + fp8 hi/lo split of the attention
  output trails off the critical path.
- PSUM bank rule honored throughout: start=True zeroes the whole 2KB bank,
  so only the first accumulation group per bank carries start=True.
"""

import numpy as np
import ml_dtypes

import concourse.bacc as bacc
import concourse.mybir as mybir
import concourse.tile as tile
from concourse.bass_utils import run_bass_kernel_spmd

F32 = mybir.dt.float32
BF16 = mybir.dt.bfloat16
F8 = mybir.dt.float8e4
AF = mybir.ActivationFunctionType
ALU = mybir.AluOpType
DR = mybir.MatmulPerfMode.DoubleRow

B, L, D = 2, 2048, 2048
HEADS, HD, R = 16, 128, 256
NC_CORES = 8
HPC = 4                  # heads per core
DQ = HPC * HD            # 512 local q/k/v dims
KC = 16                  # 128-row chunks of hidden dim
XC = 4                   # 512-token phase-1 chunks
YB = 16                  # 128-row y chunks
XB = 8                   # 256-wide attention x blocks
EPS = 1e-6
WS = 32.0                # weight pre-scale before fp8 split
ISQHD = float(1.0 / np.sqrt(HD))

F8NP = ml_dtypes.float8_e4m3
BFNP = ml_dtypes.bfloat16


def _emit(tc):
    nc = tc.nc
    xhl_d = nc.dram_tensor("xhl", [128, KC, 2, L], F8, kind="ExternalInput").ap()
    wq_d = nc.dram_tensor("wq", [128, 2, KC, DQ], F8, kind="ExternalInput").ap()
    wk_d = nc.dram_tensor("wk", [128, 2, KC, R], F8, kind="ExternalInput").ap()
    wuk_d = nc.dram_tensor("wuk", [128, 2, DQ], BF16, kind="ExternalInput").ap()
    wuv_d = nc.dram_tensor("wuv", [128, 2, DQ], BF16, kind="ExternalInput").ap()
    wo_d = nc.dram_tensor("wo", [128, 2, HPC, D], F8, kind="ExternalInput").ap()
    mska_d = nc.dram_tensor("mska", [128, HPC, 128], BF16, kind="ExternalInput").ap()
    idn_d = nc.dram_tensor("idn", [128, 128], BF16, kind="ExternalInput").ap()
    out_d = nc.dram_tensor("out", [L, D], F32, kind="ExternalOutput").ap()

    constp = tc.alloc_tile_pool(name="constp", bufs=1)
    zero_b = constp.tile([128, 1], F32, name="zero_b")
    nc.vector.memset(zero_b, 0.0)
    eps_b = constp.tile([128, 1], F32, name="eps_b")
    nc.vector.memset(eps_b, EPS)
    ones_bf = constp.tile([128, 1], BF16, name="ones_bf")
    nc.vector.memset(ones_bf, 1.0)
    oinv_bf = constp.tile([128, 1], BF16, name="oinv_bf")
    nc.vector.memset(oinv_bf, 1.0 / R)
    onef = constp.tile([1, 1], F32, name="onef")
    nc.vector.memset(onef, 1.0)

    wp = tc.alloc_tile_pool(name="wp", bufs=1)
    wq_sb = wp.tile([128, 2, KC, DQ], F8, name="wq_sb")
    wk_sb = wp.tile([128, 2, KC, R], F8, name="wk_sb")
    wuk_sb = wp.tile([128, 2, DQ], BF16, name="wuk_sb")
    wuv_sb = wp.tile([128, 2, DQ], BF16, name="wuv_sb")
    wohl_sb = wp.tile([128, 2, HPC, D], F8, name="wohl_sb")
    mska_sb = wp.tile([128, HPC, 128], BF16, name="mska_sb")
    idn_sb = wp.tile([128, 128], BF16, name="idn_sb")

    mp = tc.alloc_tile_pool(name="mp", bufs=1)
    qT = mp.tile([128, HPC, L], BF16, name="qT")
    kT = mp.tile([128, HPC, L], BF16, name="kT")
    v_sb = mp.tile([128, YB, DQ], BF16, name="v_sb")
    latT = mp.tile([128, 2, L], BF16, name="latT")
    aohl = mp.tile([128, HPC, 2, L], F8, name="aohl")
    ms_sb = mp.tile([1, L], F32, name="ms_sb")
    rstd_p = mp.tile([128, YB], F32, name="rstd_p")
    rstd_s = mp.tile([128, YB], F32, name="rstd_s")

    # ---- input DMAs, quarter-interleaved so PE can start ~3us in ----
    xq = tc.alloc_tile_pool(name="xq", bufs=1)
    xts = [None] * XC

    def x_tile(xc):
        xts[xc] = xq.tile([128, KC, 2, 512], F8, tag="x", bufs=2, name=f"x{xc}")
        return xts[xc]

    xsl = lambda xc: slice(xc * 512, (xc + 1) * 512)
    xt0 = x_tile(0)
    for qtr in range(4):
        kq = slice(4 * qtr, 4 * qtr + 4)
        nc.sync.dma_start(out=wq_sb[:, :, kq, :], in_=wq_d[:, :, kq, :])
        nc.sync.dma_start(out=xt0[:, kq, :, :], in_=xhl_d[:, kq, :, xsl(0)])
        if qtr == 1:
            nc.sync.dma_start(out=wk_sb[:, :, 0:8, :], in_=wk_d[:, :, 0:8, :])
        if qtr == 2:
            nc.sync.dma_start(out=wk_sb[:, :, 8:16, :], in_=wk_d[:, :, 8:16, :])
    xt1 = x_tile(1)
    nc.sync.dma_start(out=xt1, in_=xhl_d[:, :, :, xsl(1)])
    nc.gpsimd.dma_start(out=wuk_sb, in_=wuk_d)
    nc.gpsimd.dma_start(out=wuv_sb, in_=wuv_d)
    nc.gpsimd.dma_start(out=wohl_sb, in_=wo_d)
    nc.gpsimd.dma_start(out=mska_sb, in_=mska_d)
    nc.gpsimd.dma_start(out=idn_sb, in_=idn_d)

    # ---- phase 1: qT = (Wq.T x.T)/32, latT = (Wkd.T x.T)/32, ms ----
    psA = tc.alloc_tile_pool(name="psA", bufs=1, space="PSUM")
    smp = tc.alloc_tile_pool(name="smp", bufs=1)

    def dr_mains(ps, w_sb, osl, xt, j0, j1, first):
        for j in range(j0, j1):
            nc.tensor.matmul(ps, lhsT=w_sb[:, 1, 2 * j:2 * j + 2, osl],
                             rhs=xt[:, 2 * j:2 * j + 2, 0, :],
                             start=(first and j == j0), stop=False, perf_mode=DR)

    def dr_crosses(ps, w_sb, osl, xt, k0, k1, last):
        for k in range(k0, k1):
            nc.tensor.matmul(ps, lhsT=w_sb[:, 0:2, k, osl],
                             rhs=xt[:, k, 0:2, :],
                             start=False, stop=(last and k == k1 - 1), perf_mode=DR)

    for xc in range(XC):
        xt = xts[xc]
        if xc >= 1 and xc + 1 < XC:  # prefetch next chunk
            nxt = x_tile(xc + 1)
            nc.sync.dma_start(out=nxt, in_=xhl_d[:, :, :, xsl(xc + 1)])
        qps = [psA.tile([128, 512], F32, tag=f"q{oc}", bufs=1, name=f"q{xc}_{oc}")
               for oc in range(HPC)]
        lps = [psA.tile([128, 512], F32, tag=f"lat{rc}", bufs=1, name=f"lat{xc}_{rc}")
               for rc in range(2)]
        if xc == 0:
            # stream with quarter-granular DMA arrivals
            parts = [("q", 0, 4), ("q", 4, 8), ("lat", 0, 8),
                     ("q", 8, 12), ("q", 12, 16), ("lat", 8, 16)]
        else:
            parts = [("q", 0, 16), ("lat", 0, 16)]
        for kind, k0, k1 in parts:
            tiles = qps if kind == "q" else lps
            for i, ps in enumerate(tiles):
                osl = slice(i * 128, (i + 1) * 128)
                dr_mains(ps, wq_sb if kind == "q" else wk_sb, osl, xt,
                         k0 // 2, k1 // 2, k0 == 0)
                dr_crosses(ps, wq_sb if kind == "q" else wk_sb, osl, xt,
                           k0, k1, k1 == 16)
        for oc in range(HPC):
            nc.scalar.activation(qT[:, oc, xsl(xc)], qps[oc], AF.Copy,
                                 bias=0.0, scale=1.0 / WS)
        ms_ps = psA.tile([1, 512], F32, tag="ms", bufs=1, name=f"ms{xc}")
        for rc in range(2):
            nc.scalar.activation(latT[:, rc, xsl(xc)], lps[rc], AF.Copy,
                                 bias=0.0, scale=1.0 / WS)
            sq = smp.tile([128, 512], BF16, tag="sq", bufs=2, name=f"sq{xc}_{rc}")
            nc.scalar.activation(sq, lps[rc], AF.Square, bias=zero_b, scale=1.0 / WS)
            nc.tensor.matmul(ms_ps, lhsT=oinv_bf, rhs=sq,
                             start=(rc == 0), stop=(rc == 1), skip_group_check=True)
        nc.scalar.copy(ms_sb[0:1, xsl(xc)], ms_ps)

    # ---- rstd: transpose ms -> [128, YB], 1/sqrt(ms+eps) ----
    msT_ps = psA.tile([128, YB], F32, tag="msT", bufs=1)
    for j in range(YB):
        nc.tensor.matmul(msT_ps[:, j:j + 1],
                         lhsT=ms_sb[0:1, j * 128:(j + 1) * 128],
                         rhs=onef, start=True, stop=True, skip_group_check=True)
    t_p = smp.tile([128, YB], F32, tag="tp", bufs=1)
    nc.scalar.activation(t_p, msT_ps, AF.Sqrt, bias=eps_b, scale=1.0)
    nc.vector.reciprocal(rstd_p, t_p)
    nc.vector.tensor_scalar_mul(rstd_s, rstd_p, ISQHD)

    # ---- phase 2: kT = Wuk.T latT; v = (lat @ Wuv) * rstd ----
    for yc in range(4):
        ysl = slice(yc * 512, (yc + 1) * 512)
        for ec in range(HPC):
            ps = psA.tile([128, 512], F32, tag=f"q{ec}", bufs=1, name=f"k{yc}_{ec}")
            for rc in range(2):
                nc.tensor.matmul(ps, lhsT=wuk_sb[:, rc, ec * 128:(ec + 1) * 128],
                                 rhs=latT[:, rc, ysl], start=(rc == 0), stop=(rc == 1))
            nc.scalar.copy(kT[:, ec, ysl], ps)
        for j in range(4):
            yg = yc * 4 + j
            ps = psA.tile([128, DQ], F32, tag=f"lat{j % 2}", bufs=1, name=f"v{yg}")
            for rc in range(2):
                nc.tensor.matmul(ps, lhsT=latT[:, rc, yg * 128:(yg + 1) * 128],
                                 rhs=wuv_sb[:, rc, :], start=(rc == 0), stop=(rc == 1))
            nc.vector.tensor_scalar_mul(v_sb[:, yg, :], ps, rstd_p[:, yg:yg + 1])
    psA.release()
    smp.release()
    xq.release()

    # ---- attention: 256-wide x blocks, causal ----
    psC = tc.alloc_tile_pool(name="psC", bufs=1, space="PSUM")
    atp = tc.alloc_tile_pool(name="atp", bufs=1)
    for xb in range(XB):
        x0 = xb * 256
        nyc = 2 * xb + 2
        ao_ps = psC.tile([128, HPC, 256], F32, tag="ao", bufs=1, name=f"ao{xb}")
        l_ps = psC.tile([1, HPC, 256], F32, tag="l", bufs=1, name=f"l{xb}")

        def emit_scores(iy):
            st = psC.tile([128, HPC, 256], F32, tag="st", bufs=2,
                          name=f"st{xb}_{iy}")
            d1 = iy == 2 * xb
            d2 = iy == 2 * xb + 1
            ks = slice(iy * 128, (iy + 1) * 128)
            if d2:
                for h in range(HPC):
                    nc.tensor.matmul(st[:, h, 128:256], lhsT=kT[:, h, ks],
                                     rhs=qT[:, h, x0 + 128:x0 + 256],
                                     start=(h % 2 == 0), stop=False,
                                     skip_group_check=True)
                for h in range(HPC):
                    nc.tensor.matmul(st[:, h, 128:256], lhsT=idn_sb,
                                     rhs=mska_sb[:, h, :],
                                     start=False, stop=True, skip_group_check=True)
            elif d1:
                for h in range(HPC):
                    nc.tensor.matmul(st[:, h, 0:128], lhsT=kT[:, h, ks],
                                     rhs=qT[:, h, x0:x0 + 128],
                                     start=(h % 2 == 0), stop=False,
                                     skip_group_check=True)
                    nc.tensor.matmul(st[:, h, 128:256], lhsT=kT[:, h, ks],
                                     rhs=qT[:, h, x0 + 128:x0 + 256],
                                     start=False, stop=True, skip_group_check=True)
                for h in range(HPC):
                    nc.tensor.matmul(st[:, h, 0:128], lhsT=idn_sb,
                                     rhs=mska_sb[:, h, :],
                                     start=False, stop=True, skip_group_check=True)
            else:
                for h in range(HPC):
                    nc.tensor.matmul(st[:, h, :], lhsT=kT[:, h, ks],
                                     rhs=qT[:, h, x0:x0 + 256],
                                     start=(h % 2 == 0), stop=True,
                                     skip_group_check=True)
            return st

        def emit_rest(iy, st):
            d2 = iy == 2 * xb + 1
            at = atp.tile([128, HPC, 256], BF16, tag="at", bufs=5,
                          name=f"at{xb}_{iy}")
            if d2:
                nc.scalar.activation(at[:, :, 128:256], st[:, :, 128:256], AF.Exp,
                                     bias=zero_b, scale=rstd_s[:, iy:iy + 1])
                for h in range(HPC):
                    nc.tensor.matmul(ao_ps[:, h, 128:256],
                                     lhsT=v_sb[:, iy, h * 128:(h + 1) * 128],
                                     rhs=at[:, h, 128:256],
                                     start=False, stop=(iy == nyc - 1),
                                     skip_group_check=True)
                for h in range(HPC):
                    nc.tensor.matmul(l_ps[0:1, h, 128:256], lhsT=ones_bf,
                                     rhs=at[:, h, 128:256],
                                     start=False, stop=(iy == nyc - 1),
                                     skip_group_check=True)
            else:
                nc.scalar.activation(at, st, AF.Exp,
                                     bias=zero_b, scale=rstd_s[:, iy:iy + 1])
                for h in range(HPC):
                    nc.tensor.matmul(ao_ps[:, h, :],
                                     lhsT=v_sb[:, iy, h * 128:(h + 1) * 128],
                                     rhs=at[:, h, :],
                                     start=(iy == 0 and h % 2 == 0),
                                     stop=(iy == nyc - 1), skip_group_check=True)
                for hp in range(2):
                    nc.tensor.matmul(l_ps[0:1, 2 * hp:2 * hp + 2, :], lhsT=ones_bf,
                                     rhs=at[:, 2 * hp:2 * hp + 2, :],
                                     start=(iy == 0), stop=(iy == nyc - 1),
                                     skip_group_check=True)

        # scores one iy ahead of exp/AV so the PE always has queued work
        st_cur = emit_scores(0)
        for iy in range(nyc):
            st_nxt = emit_scores(iy + 1) if iy + 1 < nyc else None
            emit_rest(iy, st_cur)
            st_cur = st_nxt

        # block tail: free ao (plain copies) and l (broadcast) fast, then
        # normalize + fp8 hi/lo split off the critical path
        tbf = atp.tile([128, HPC, 256], BF16, tag="tb", bufs=2, name=f"tb{xb}")
        for hp in range(2):
            nc.vector.tensor_copy(tbf[:, 2 * hp:2 * hp + 2, :],
                                  ao_ps[:, 2 * hp:2 * hp + 2, :])
        lb = atp.tile([128, HPC, 256], F32, tag="lb", bufs=2, name=f"lb{xb}")
        nc.gpsimd.partition_broadcast(lb, l_ps)
        rlb = atp.tile([128, HPC, 256], F32, tag="rlb", bufs=2, name=f"rlb{xb}")
        nc.vector.reciprocal(rlb, lb)
        nbf = atp.tile([128, HPC, 256], BF16, tag="nb", bufs=2, name=f"nb{xb}")
        for h in range(HPC):
            nc.vector.tensor_mul(nbf[:, h, :], tbf[:, h, :], rlb[:, h, :])
        for h in range(HPC):
            nc.scalar.activation(aohl[:, h, 0, x0:x0 + 256], nbf[:, h, :],
                                 AF.Copy, bias=0.0, scale=1.0)
        for h in range(HPC):
            nc.vector.tensor_sub(aohl[:, h, 1, x0:x0 + 256], nbf[:, h, :],
                                 aohl[:, h, 0, x0:x0 + 256])
    psC.release()
    atp.release()

    # ---- o_proj (fp8 DR): y[x, d] = sum_h ao[:, h, x].T @ wo[:, h, d] ----
    psD = tc.alloc_tile_pool(name="psD", bufs=1, space="PSUM")
    op = tc.alloc_tile_pool(name="op", bufs=1)
    for xs in range(16):
        o = op.tile([128, D], F32, tag="o", bufs=2, name=f"o{xs}")
        for mc in range(4):
            msl = slice(mc * 512, (mc + 1) * 512)
            wps = psD.tile([128, 512], F32, tag="w", bufs=4, name=f"w{xs}_{mc}")
            xsl2 = slice(xs * 128, (xs + 1) * 128)
            for u in range(2):  # main hi*hi over head pairs
                nc.tensor.matmul(wps, lhsT=aohl[:, 2 * u:2 * u + 2, 0, xsl2],
                                 rhs=wohl_sb[:, 1, 2 * u:2 * u + 2, msl],
                                 start=(u == 0), stop=False, perf_mode=DR)
            for h in range(HPC):  # cross hi*lo + lo*hi per head
                nc.tensor.matmul(wps, lhsT=aohl[:, h, 0:2, xsl2],
                                 rhs=wohl_sb[:, 0:2, h, msl],
                                 start=False, stop=(h == HPC - 1), perf_mode=DR)
            if mc % 2 == 0:
                nc.scalar.activation(o[:, msl], wps, AF.Copy,
                                     bias=0.0, scale=1.0 / WS)
            else:
                nc.vector.tensor_scalar_mul(o[:, msl], wps, 1.0 / WS)
        nc.sync.dma_start(out=out_d[xs * 128:(xs + 1) * 128, :], in_=o)
    psD.release()
    op.release()
    mp.release()
    wp.release()
    constp.release()


_NC_CACHE = None


def _build():
    global _NC_CACHE
    if _NC_CACHE is None:
        nc = bacc.Bacc()
        with tile.TileContext(nc) as tc:
            _emit(tc)
        nc.compile()
        _NC_CACHE = nc
    return _NC_CACHE


def _hl(a):
    """fp8 hi/lo error-compensation split of a float32 array."""
    hi = a.astype(F8NP)
    lo = (a - hi.astype(np.float32)).astype(F8NP)
    return hi, lo


def make_in_maps(inputs):
    x = np.asarray(inputs["x"], np.float32)
    Wq = np.asarray(inputs["Wq"], np.float32)
    Wkd = np.asarray(inputs["Wkv_down"], np.float32)
    Wup = np.asarray(inputs["Wkv_up"], np.float32) * np.asarray(
        inputs["kv_norm_w"], np.float32)[:, None]
    Wo = np.asarray(inputs["Wo"], np.float32)

    xhl_b = []
    for b in range(B):
        xT = np.ascontiguousarray(x[b].T).reshape(KC, 128, L).transpose(1, 0, 2)
        hi, lo = _hl(xT)
        xhl_b.append(np.ascontiguousarray(np.stack([hi, lo], axis=2)))

    def wpack(W, nchunk, ncol):  # [K, ncol] f32 -> [128, 2(lo,hi), nchunk, ncol]
        Wt = W.reshape(nchunk, 128, ncol).transpose(1, 0, 2)
        hi, lo = _hl(Wt)
        return np.ascontiguousarray(np.stack([lo, hi], axis=1))

    parts = np.arange(128)
    mask = np.where(np.arange(128)[None, None, :] < parts[:, None, None],
                    np.float32(-30000.0), np.float32(0.0))
    mask = np.broadcast_to(mask, (128, HPC, 128))

    wq_g = [wpack(Wq[:, g * DQ:(g + 1) * DQ] * WS, KC, DQ) for g in range(HPC)]
    wk_p = wpack(Wkd * WS, KC, R)
    wo_g = [wpack(Wo[g * DQ:(g + 1) * DQ, :] * WS, HPC, D) for g in range(HPC)]
    in_maps = []
    for c in range(NC_CORES):
        b, g = divmod(c, HPC)
        wuk = Wup[:, g * DQ:(g + 1) * DQ]
        wuv = Wup[:, HEADS * HD + g * DQ:HEADS * HD + (g + 1) * DQ]
        in_maps.append({
            "xhl": xhl_b[b],
            "wq": wq_g[g],
            "wk": wk_p,
            "wuk": np.ascontiguousarray(
                wuk.reshape(2, 128, DQ).transpose(1, 0, 2)).astype(BFNP),
            "wuv": np.ascontiguousarray(
                wuv.reshape(2, 128, DQ).transpose(1, 0, 2)).astype(BFNP),
            "wo": wo_g[g],
            "mska": np.ascontiguousarray(mask).astype(BFNP),
            "idn": np.eye(128, dtype=np.float32).astype(BFNP),
        })
    return in_maps


def kernel(x, Wq, Wkv_down, kv_norm_w, Wkv_up, Wo):
    in_maps = make_in_maps(dict(x=x, Wq=Wq, Wkv_down=Wkv_down,
                                kv_norm_w=kv_norm_w, Wkv_up=Wkv_up, Wo=Wo))
    nc = _build()
    res = run_bass_kernel_spmd(nc, in_maps, core_ids=list(range(NC_CORES)))
    outs = [r["out"].astype(np.float32) for r in res.results]
    y0 = outs[0] + outs[1] + outs[2] + outs[3]
    y1 = outs[4] + outs[5] + outs[6] + outs[7]
    return np.stack([y0, y1]).reshape(B, L, D)
